# revision 33
# baseline (speedup 1.0000x reference)
"""EventSpecificTimingHeads Trainium2 kernel (8 NeuronCores, SPMD), v2.

Shards the E=16 independent per-event attention+MLP heads across 8 cores
(2 events per core). Design is Activation-engine-bound (softmax exp is the
fundamental wall: 16.8M exps/core ~= 109us minimum on ACT), so everything
else is arranged to hide under it:

  - QK^T emitted as contiguous 4-head quads (tile_position row-packing ->
    concurrent execution on the PE's 32-row sub-arrays).
  - PV ones-augmented (l rides as the 33rd row), 2-way col-packed pairs.
  - One exp unit per instance ([128,1024] of 8) computed on the Vector
    engine via the Schraudolph bit-trick written directly as bf16 bits
    (int16 view), offloading ~12.5% of the exp wall.
  - Wo and W1 folded into one host-precomputed Wc = W1 @ Wo matmul.
  - Per-head softmax normalization via the transpose dance (PE transposes
    + per-partition reciprocal/multiply), then Wc / relu(+c1) / W2-aug.
  - Epilogue of instance n-1 interleaved into instance n's engine queues.
"""
import sys

if "/opt/trn_rl_repo" not in sys.path:
    sys.path.insert(0, "/opt/trn_rl_repo")

import numpy as np
import ml_dtypes

import concourse.bass as bass
import concourse.bacc as bacc
import concourse.tile as tile
from concourse import mybir
from concourse import masks
from concourse.bass_utils import run_bass_kernel_spmd

BF16 = mybir.dt.bfloat16
F32 = mybir.dt.float32
I16 = mybir.dt.int16
AF = mybir.ActivationFunctionType
ALU = mybir.AluOpType

E, D, B, S, H, Dh, H2 = 16, 128, 8, 512, 4, 32, 64
T = B * S            # 4096
EV = 2               # events per core
NCORES = 8

# Schraudolph exp -> bf16 bits via int16:  bits = round(x*C1 + C2)
# C1 = 2^7/ln2 ; C2 = 127*2^7 - 366393/65536 (minmax-tuned fp32 constant
# scaled to the bf16 exponent/mantissa grid).
SCH_C1 = 184.6650292
SCH_C2 = 16250.4091
# which (jc, hp) exp units run on the Vector engine (per instance)
SCHRAUD_UNITS = ((0, 1), (1, 1), (3, 1))

_CACHED_NC = None


def build_nc():
    nc = bacc.Bacc(None, target_bir_lowering=False, debug=False)

    xT_d = nc.declare_dram_parameter("xT", [D, T], BF16, isOutput=False)
    wqkvT_d = nc.declare_dram_parameter("wqkvT", [D, EV, 3, D], BF16, isOutput=False)
    bqk_d = nc.declare_dram_parameter("bqk", [D, EV, 2], F32, isOutput=False)
    wcT_d = nc.declare_dram_parameter("wcT", [D, EV, H2], BF16, isOutput=False)
    c1b_d = nc.declare_dram_parameter("c1b", [H2, EV], F32, isOutput=False)
    w2a_d = nc.declare_dram_parameter("w2a", [H2 + 1, EV], BF16, isOutput=False)
    out_d = nc.declare_dram_parameter("out", [EV, B, S], F32, isOutput=True)

    with tile.TileContext(nc) as tc:
        with (
            tc.tile_pool(name="single", bufs=1) as single,
            tc.tile_pool(name="work", bufs=2) as work,
            tc.tile_pool(name="stp", bufs=2, space="PSUM") as stp,
            tc.tile_pool(name="pvp", bufs=1, space="PSUM") as pvp,
            tc.tile_pool(name="epp", bufs=1, space="PSUM") as epp,
        ):
            # ---- resident SBUF tensors ----
            xT_sb = single.tile([D, T], BF16)
            wqkvT_sb = single.tile([D, EV, 3, D], BF16)
            bqk_sb = single.tile([D, EV, 2], F32)
            wcT_sb = single.tile([D, EV, H2], BF16)
            c1b_sb = single.tile([H2, EV], F32)
            w2a_sb = single.tile([H2 + 1, EV], BF16)
            ident = single.tile([D, D], BF16)
            qT_sb = single.tile([D, EV, T], BF16)
            kT_sb = single.tile([D, EV, T], BF16)
            # v_aug per (ev, b, jc, h): [j, 33]; col 32 = 1.0 (l row source)
            v_sb = single.tile([D, EV, B, 4, H, Dh + 1], BF16)
            h1a_sb = single.tile([H2 + 1, 2, S], BF16)  # row 64 = ones

            masks.make_identity(nc, ident[:])
            nc.sync.dma_start(out=xT_sb[:, 0:D], in_=xT_d[:, 0:D])
            nc.sync.dma_start(out=xT_sb[:, D:S], in_=xT_d[:, D:S])
            nc.sync.dma_start(out=wqkvT_sb[:], in_=wqkvT_d[:])
            nc.sync.dma_start(out=bqk_sb[:], in_=bqk_d[:])
            for n in range(1, 8):
                nc.scalar.dma_start(out=xT_sb[:, n * S:(n + 1) * S],
                                    in_=xT_d[:, n * S:(n + 1) * S])
            nc.sync.dma_start(out=wcT_sb[:], in_=wcT_d[:])
            nc.sync.dma_start(out=c1b_sb[:], in_=c1b_d[:])
            nc.sync.dma_start(out=w2a_sb[:], in_=w2a_d[:])
            nc.gpsimd.memset(v_sb[:, :, :, :, :, Dh:Dh + 1], 1.0)
            nc.gpsimd.memset(h1a_sb[H2:H2 + 1, :, :], 1.0)

            # ---------- projection pieces for batch b ----------
            # Each piece = 1 matmul + 1 drain in a dedicated psum bank so
            # projections never contend with the score-tile rotation.
            def proj_pieces(b, ptag="prj"):
                t0 = b * S
                pieces = []

                def qk_piece(ev, qk):
                    def run():
                        dst = qT_sb if qk == 0 else kT_sb
                        pool = stp if ptag == "st" else epp
                        ps = pool.tile([D, 2 * S] if ptag == "st" else [D, S],
                                       F32, name="prj", tag=ptag,
                                       padded_shape=None)
                        nc.tensor.matmul(
                            ps[:, 0:S],
                            wqkvT_sb[:, ev, qk, :],
                            xT_sb[:, t0:t0 + S],
                        )
                        nc.scalar.activation(
                            dst[:, ev, t0:t0 + S], ps[:, 0:S],
                            AF.Identity,
                            bias=bqk_sb[:, ev, qk:qk + 1],
                        )
                    return run

                def v_piece(jc):
                    def run():
                        tch = 4 * b + jc
                        pool = stp if ptag == "st" else epp
                        psv = pool.tile([D, 2 * S] if ptag == "st" else [D, S],
                                        F32, name="prj", tag=ptag,
                                        padded_shape=None)
                        nc.tensor.matmul(
                            psv[:, 0:EV * D],
                            xT_sb[:, tch * D:(tch + 1) * D],
                            wqkvT_sb[:, :, 2, :],
                        )
                        nc.vector.tensor_copy(
                            v_sb[:, :, b, jc, :, 0:Dh],
                            psv[:, 0:EV * D].rearrange(
                                "p (e h d) -> p e h d", e=EV, h=H),
                        )
                    return run

                for ev in range(EV):
                    for qk in range(2):
                        pieces.append(qk_piece(ev, qk))
                for jc in range(4):
                    pieces.append(v_piece(jc))
                return pieces

            # ---------- per-instance state ----------
            class Inst:
                pass

            def qk_quad(st_tiles, ev, b, jc):
                t0 = b * S
                for h in range(H):
                    nc.tensor.matmul(
                        st_tiles[h // 2][:, (h % 2) * S:(h % 2 + 1) * S],
                        kT_sb[32 * h:32 * h + 32, ev,
                              t0 + jc * D:t0 + (jc + 1) * D],
                        qT_sb[32 * h:32 * h + 32, ev, t0:t0 + S],
                        tile_position=(32 * h, 0),
                    )

            def exp_unit(inst, st_tiles, jc, hp):
                dst = inst.pt[:, jc, 2 * hp:2 * hp + 2, :]
                if (jc, hp) in SCHRAUD_UNITS:
                    nc.vector.tensor_scalar(
                        dst.bitcast(I16),
                        st_tiles[hp][:],
                        SCH_C1, SCH_C2, ALU.mult, ALU.add,
                    )
                else:
                    nc.scalar.activation(dst, st_tiles[hp][:], AF.Exp)

            def pv_pair(inst, ev, b, jc, half):
                for s2 in range(2):
                    h = 2 * half + s2
                    nc.tensor.matmul(
                        inst.pv[64 * s2:64 * s2 + 33,
                                half * S:(half + 1) * S],
                        v_sb[:, ev, b, jc, h, :],
                        inst.pt[:, jc, h, :],
                        start=(jc == 0), stop=(jc == 3),
                        tile_position=(0, 64 * s2),
                    )

            # ---------- epilogue pieces for a finished instance ----------
            def epi_tfwd(p):
                # pv_sb [d, (half, i)] -> ct [i, (it, half, s2, 64)]
                p.ct = pvp.tile([D, 2 * 4 * D], BF16, name="ct", tag="pv",
                                padded_shape=[D, 2 * 4 * D])
                for it in range(4):
                    for half in range(2):
                        nc.tensor.transpose(
                            p.ct[:, (it * 2 + half) * D:(it * 2 + half + 1) * D],
                            p.pv_sb[:, half * S + it * D:half * S + (it + 1) * D],
                            ident[:],
                        )

            def epi_norm(p):
                ctv = p.ct[:].rearrange("p (b c) -> p b c", b=8)
                nc.vector.reciprocal(
                    p.linv[:].rearrange("p (b l) -> p b l", b=8),
                    ctv[:, :, Dh::2 * Dh],
                )
                for half in range(2):
                    nc.vector.tensor_tensor(
                        p.ctxn[:, :, 64 * half:64 * half + 64].rearrange(
                            "p it (s d) -> p it s d", s=2),
                        p.ct[:].rearrange(
                            "p (it hf s c) -> p it hf s c", it=4, hf=2, s=2
                        )[:, :, half, :, 0:Dh],
                        p.linv[:].rearrange(
                            "p (it hf s) -> p it hf s", hf=2, s=2
                        )[:, :, half, :, None].to_broadcast([D, 4, 2, Dh]),
                        ALU.mult,
                    )

            def epi_tback(p):
                p.ctp = epp.tile([D, S], BF16, name="ctp", tag="ep",
                                 padded_shape=[D, S])
                for it in range(4):
                    nc.tensor.transpose(
                        p.ctp[:, it * D:(it + 1) * D],
                        p.ctxn[:, it, :],
                        ident[:],
                    )

            def epi_ctxT(p):
                nc.scalar.activation(p.ctxT[:], p.ctp[:], AF.Copy)

            def epi_wc(p):
                p.gp = epp.tile([H2, S], F32, name="gp", tag="ep",
                                padded_shape=[H2, S])
                nc.tensor.matmul(p.gp[:], wcT_sb[:, p.ev, :], p.ctxT[:])

            def epi_relu(p):
                nc.scalar.activation(
                    h1a_sb[0:H2, p.slot, :], p.gp[:], AF.Relu,
                    bias=c1b_sb[:, p.ev:p.ev + 1],
                )

            def epi_w2(p):
                p.lgp = epp.tile([1, S], F32, name="lgp", tag="ep",
                                 padded_shape=[1, S])
                nc.tensor.matmul(
                    p.lgp[:], w2a_sb[:, p.ev:p.ev + 1], h1a_sb[:, p.slot, :]
                )

            def epi_out(p):
                lg_sb = work.tile([1, S], F32, name="lg_sb")
                nc.scalar.activation(lg_sb[:], p.lgp[:], AF.Copy)
                nc.sync.dma_start(out=out_d[p.ev, p.b, :], in_=lg_sb[0:1, :])

            # ---------- one attention instance, with prev's epilogue ----
            def instance(ev, b, slot, prev, filler):
                inst = Inst()
                inst.ev, inst.b, inst.slot = ev, b, slot
                inst.pt = work.tile([D, 4, H, S], BF16, name="pt", tag="pt")
                inst.linv = work.tile([D, 16], F32, name="linv", tag="linv")
                inst.ctxn = work.tile([D, 4, D], BF16, name="ctxn", tag="ctxn")
                inst.ctxT = work.tile([D, S], BF16, name="ctxT", tag="ctxT")

                def quad_and_exp(jc):
                    sts = [stp.tile([D, 2 * S], F32, name=f"st{hp}", tag="st")
                           for hp in range(2)]
                    qk_quad(sts, ev, b, jc)
                    exp_unit(inst, sts, jc, 0)
                    exp_unit(inst, sts, jc, 1)

                def fill(n):
                    for _ in range(n):
                        if filler:
                            filler.pop(0)()

                quad_and_exp(0)
                fill(1)
                if prev is not None:
                    epi_tfwd(prev)
                    epi_norm(prev)
                quad_and_exp(1)
                # PV psum: [rows: s2-bands, cols: half*S]
                inst.pv = pvp.tile([D, 2 * S], F32, name="pv", tag="pv",
                                   padded_shape=[D, 2 * S])
                pv_pair(inst, ev, b, 0, 0)
                pv_pair(inst, ev, b, 0, 1)
                fill(1)
                if prev is not None:
                    epi_tback(prev)
                    epi_ctxT(prev)
                quad_and_exp(2)
                pv_pair(inst, ev, b, 1, 0)
                pv_pair(inst, ev, b, 1, 1)
                fill(1)
                if prev is not None:
                    epi_wc(prev)
                quad_and_exp(3)
                if prev is not None:
                    epi_relu(prev)
                pv_pair(inst, ev, b, 2, 0)
                pv_pair(inst, ev, b, 2, 1)
                if prev is not None:
                    epi_w2(prev)
                    epi_out(prev)
                pv_pair(inst, ev, b, 3, 0)
                # drain PV halves to SBUF (bf16) as they complete
                inst.pv_sb = work.tile([D, 2 * S], BF16, name="pv_sb",
                                       tag="pv_sb")
                nc.vector.tensor_copy(inst.pv_sb[:, 0:S], inst.pv[:, 0:S])
                pv_pair(inst, ev, b, 3, 1)
                nc.vector.tensor_copy(inst.pv_sb[:, S:2 * S],
                                      inst.pv[:, S:2 * S])
                fill(1)
                return inst

            # ---------- main loop ----------
            for piece in proj_pieces(0, ptag="st"):
                piece()
            prev = None
            idx = 0
            for b in range(B):
                filler = proj_pieces(b + 1) if b + 1 < B else []
                prev = instance(0, b, idx % 2, prev, filler)
                idx += 1
                prev = instance(1, b, idx % 2, prev, filler)
                idx += 1
                assert not filler
            # tail epilogue
            epi_tfwd(prev)
            epi_norm(prev)
            epi_tback(prev)
            epi_ctxT(prev)
            epi_wc(prev)
            epi_relu(prev)
            epi_w2(prev)
            epi_out(prev)

    nc.compile()
    return nc


def _prep_inputs(lstm_features, Wqkv, bqkv, Wo, bo, W1, b1, W2, b2):
    """Host-side per-core input prep (numpy, fp32 -> bf16 where PE-facing)."""
    bf = ml_dtypes.bfloat16
    x = np.asarray(lstm_features, np.float32).reshape(T, D)
    xT = np.ascontiguousarray(x.T).astype(bf)
    scale = 1.0 / np.sqrt(np.float32(Dh))

    in_maps = []
    for c in range(NCORES):
        evs = [2 * c, 2 * c + 1]
        wqkvT = np.zeros((D, EV, 3, D), np.float32)
        bqk = np.zeros((D, EV, 2), np.float32)
        wcT = np.zeros((D, EV, H2), np.float32)
        c1b = np.zeros((H2, EV), np.float32)
        w2a = np.zeros((H2 + 1, EV), np.float32)
        for i, e in enumerate(evs):
            Wq = Wqkv[e, 0:D, :] * scale
            Wk = Wqkv[e, D:2 * D, :]
            Wv = Wqkv[e, 2 * D:3 * D, :]
            wqkvT[:, i, 0, :] = Wq.T
            wqkvT[:, i, 1, :] = Wk.T
            wqkvT[:, i, 2, :] = Wv.T
            bqk[:, i, 0] = bqkv[e, 0:D] * scale
            bqk[:, i, 1] = bqkv[e, D:2 * D]
            bv = bqkv[e, 2 * D:3 * D]
            bo_eff = Wo[e] @ bv + bo[e]
            wcT[:, i, :] = (W1[e] @ Wo[e]).T
            c1b[:, i] = W1[e] @ bo_eff + b1[e]
            w2a[0:H2, i] = W2[e, 0, :]
            w2a[H2, i] = b2[e, 0]
        in_maps.append({
            "xT": xT,
            "wqkvT": wqkvT.astype(bf),
            "bqk": bqk,
            "wcT": wcT.astype(bf),
            "c1b": c1b,
            "w2a": w2a.astype(bf),
        })
    return in_maps


def kernel(lstm_features, Wqkv, bqkv, Wo, bo, W1, b1, W2, b2, _trace=False):
    global _CACHED_NC
    args = [np.asarray(a, np.float32) for a in
            (lstm_features, Wqkv, bqkv, Wo, bo, W1, b1, W2, b2)]
    in_maps = _prep_inputs(*args)
    if _CACHED_NC is None:
        _CACHED_NC = build_nc()
    res = run_bass_kernel_spmd(
        _CACHED_NC, in_maps, list(range(NCORES)), trace=_trace
    )
    logits = np.concatenate(
        [np.asarray(res.results[c]["out"], np.float32) for c in range(NCORES)],
        axis=0,
    )  # [16, 8, 512]
    out = np.ascontiguousarray(logits.transpose(1, 2, 0))  # [B, S, E]
    if _trace:
        return out, res
    return out
